# revision 6
# baseline (speedup 1.0000x reference)
"""Biased multi-head cross-attention on 8 TRN2 NeuronCores.

Math (per reference): q/k/v linear projections, scores = q@k^T/sqrt(hd) + bias,
softmax over source dim, attn = probs@v, then the "faithful" reshape
(B,H,T,hd)->(B,T,E) WITHOUT transposing heads back, followed by the out
projection. That reshape makes output rows [h*64,(h+1)*64) of each batch
depend only on head h, so the whole problem is embarrassingly parallel over
(batch, head): core c owns batch c//2 and heads (c%2)*8..(c%2)*8+8, computes
its 512 output rows, and the host concatenates. No collectives.

Per-core layout strategy:
  - All matmul operands pre-transposed once on PE so every projection
    contracts E on partitions (hsT/kvT/W*T, bf16).
  - q^T/k^T stored packed two-heads-per-128-partitions; the K=64 score
    matmuls of an even/odd head pair target PE row-groups 0-63/64-127
    (tile_position) so they can run concurrently.
  - scoresT[s,t] per (head, t-block 512, s-tile 128): bias tile is DMA'd
    naturally [t,s], PE-transposed f32 into the PSUM bank, then the K^T.Q
    matmul accumulates on top (start=False). ACT applies exp straight from
    PSUM into SBUF bf16 (no max-subtraction: scores are O(+-10), softmax
    shift-invariance makes this safe in f32).
  - attnT[d,t] = sum_s v_aug[s,d]*pT[s,t] with v augmented by a ones column,
    so row 64 of the PSUM accumulator is the softmax denominator for free.
  - attnT is transposed back to [t,65], rows scaled by 1/denominator,
    written bf16 to DRAM scratch, re-read as [64,1024] (the faithful reshape
    is a pure memory view), transposed into the stationary operand of the
    out-projection against WoT.
"""

import sys

for _p in ("/opt/trn_rl_repo", "/root/.axon_site/_ro/trn_rl_repo"):
    if _p not in sys.path:
        sys.path.insert(0, _p)

import numpy as np

B, T, S, E = 4, 1024, 2048, 1024
H, HD = 16, 64
NCORES = 8
NH = 8          # heads per core
P = 128
TB = T // 512   # t-blocks of 512
NSI = S // P    # s-tiles of 128
scaling = HD ** -0.5

_cache = {}


def _build_nc():
    import concourse.mybir as mybir
    import concourse.tile as tile
    from concourse import bacc
    from concourse.masks import make_identity

    f32 = mybir.dt.float32
    bf16 = mybir.dt.bfloat16
    Exp = mybir.ActivationFunctionType.Exp
    Ident = mybir.ActivationFunctionType.Identity

    nc = bacc.Bacc(None, target_bir_lowering=False)

    hs_p = nc.declare_dram_parameter("hs", [T, E], f32, isOutput=False)
    kv_p = nc.declare_dram_parameter("kv", [S, E], f32, isOutput=False)
    bias_p = nc.declare_dram_parameter("bias", [NH, T, S], f32, isOutput=False)
    Wq_p = nc.declare_dram_parameter("Wq", [NH * HD, E], f32, isOutput=False)
    Wk_p = nc.declare_dram_parameter("Wk", [NH * HD, E], f32, isOutput=False)
    Wv_p = nc.declare_dram_parameter("Wv", [NH * HD, E], f32, isOutput=False)
    Wo_p = nc.declare_dram_parameter("Wo", [E, E], f32, isOutput=False)
    bq_p = nc.declare_dram_parameter("bq", [NH * HD], f32, isOutput=False)
    bk_p = nc.declare_dram_parameter("bk", [NH * HD], f32, isOutput=False)
    bv_p = nc.declare_dram_parameter("bv", [NH * HD], f32, isOutput=False)
    bo_p = nc.declare_dram_parameter("bo", [E], f32, isOutput=False)
    out_p = nc.declare_dram_parameter("out", [NH * HD, E], f32, isOutput=True)

    with tile.TileContext(nc) as tc:
        with tc.tile_pool(name="statics", bufs=1) as statics, \
             tc.tile_pool(name="dram", bufs=1, space="DRAM") as dram:
            id_f32 = statics.tile([P, P], f32)
            make_identity(nc, id_f32[:])
            id_bf = statics.tile([P, P], bf16)
            make_identity(nc, id_bf[:])

            scratch = dram.tile([NH, T, HD], bf16)

            # persistent bf16 operands
            WqT = statics.tile([P, 8, NH * HD], bf16)   # [E-part, Echunk, 512]
            WkT = statics.tile([P, 8, NH * HD], bf16)
            WvT = statics.tile([P, 8, NH * HD], bf16)
            WoT = statics.tile([P, 8, E], bf16)
            # head hl lives at partitions (hl%2)*64..+64, index hl//2
            qT = statics.tile([P, NH // 2, T], bf16)
            kT = statics.tile([P, NH // 2, S], bf16)
            v_aug = statics.tile([P, NH, NSI, HD + 1], bf16)  # [s-part, head, s-tile, d+1]
            # bq_s[p, hp] = scaled bq for head hp*2 + p//64, dim p%64
            bq_s = statics.tile([P, NH // 2], f32)
            bk_s = statics.tile([P, NH // 2], f32)
            bv_row = statics.tile([1, NH * HD], f32)
            bv_rep = statics.tile([P, NH, HD], f32)
            bo_row = statics.tile([1, E], f32)
            bo_rep = statics.tile([P, E], f32)

            # ---------------- phase 1: transposes + projections ----------------
            with tc.tile_pool(name="p1_big", bufs=1) as p1_big, \
                 tc.tile_pool(name="p1_stage", bufs=3) as stage_pool, \
                 tc.tile_pool(name="p1_tp", bufs=3, space="PSUM") as tp_pool, \
                 tc.tile_pool(name="p1_qk", bufs=2, space="PSUM") as qk_pool, \
                 tc.tile_pool(name="p1_v", bufs=2, space="PSUM") as v_pool:

                # small bias vectors
                nc.sync.dma_start(bq_s[:], bq_p.rearrange("(hp p) -> p hp", p=P))
                nc.vector.tensor_scalar_mul(bq_s[:], bq_s[:], scaling)
                nc.sync.dma_start(bk_s[:], bk_p.rearrange("(hp p) -> p hp", p=P))
                nc.sync.dma_start(bv_row[:], bv_p[None, :])
                nc.sync.dma_start(bo_row[:], bo_p[None, :])
                for hl in range(NH):
                    nc.gpsimd.partition_broadcast(
                        bv_rep[:, hl, :], bv_row[0:1, hl * HD:(hl + 1) * HD])
                nc.gpsimd.partition_broadcast(bo_rep[:], bo_row[0:1, :])
                nc.any.memset(v_aug[:, :, :, HD:HD + 1], 1.0)

                def transpose_in(dst, src_handle, rows):
                    # src [rows, E] f32 DRAM -> dst [128, 8, rows] bf16 (E on partitions)
                    for i in range(rows // P):
                        st = stage_pool.tile([P, E], f32, tag="st")
                        nc.sync.dma_start(st[:], src_handle[i * P:(i + 1) * P, :])
                        for j in range(8):
                            tp = tp_pool.tile([P, P], f32, tag="tp")
                            nc.tensor.transpose(tp[:], st[:, j * P:(j + 1) * P], id_f32[:])
                            nc.vector.tensor_copy(dst[:, j, i * P:(i + 1) * P], tp[:])

                hsT = p1_big.tile([P, 8, T], bf16)
                kvT = p1_big.tile([P, 8, S], bf16)
                transpose_in(hsT, hs_p, T)
                transpose_in(kvT, kv_p, S)
                transpose_in(WqT, Wq_p, NH * HD)
                transpose_in(WkT, Wk_p, NH * HD)
                transpose_in(WvT, Wv_p, NH * HD)
                transpose_in(WoT, Wo_p, E)

                halves = (slice(0, HD), slice(HD, P))
                for hp in range(NH // 2):
                    hsl = (slice(hp * P, hp * P + HD), slice(hp * P + HD, hp * P + P))
                    for tb in range(TB):
                        ps = qk_pool.tile([P, 512], f32, tag="qk")
                        for g in range(2):
                            for j in range(8):
                                nc.tensor.matmul(
                                    ps[halves[g], :], WqT[:, j, hsl[g]],
                                    hsT[:, j, tb * 512:(tb + 1) * 512],
                                    start=(j == 0), stop=(j == 7))
                            nc.scalar.activation(
                                qT[halves[g], hp, tb * 512:(tb + 1) * 512],
                                ps[halves[g], :], Ident,
                                bias=bq_s[halves[g], hp:hp + 1], scale=scaling)
                    for sb in range(S // 512):
                        ps = qk_pool.tile([P, 512], f32, tag="qk")
                        for g in range(2):
                            for j in range(8):
                                nc.tensor.matmul(
                                    ps[halves[g], :], WkT[:, j, hsl[g]],
                                    kvT[:, j, sb * 512:(sb + 1) * 512],
                                    start=(j == 0), stop=(j == 7))
                            nc.scalar.activation(
                                kT[halves[g], hp, sb * 512:(sb + 1) * 512],
                                ps[halves[g], :], Ident,
                                bias=bk_s[halves[g], hp:hp + 1])
                for hl in range(NH):
                    for si in range(NSI):
                        ps = v_pool.tile([P, HD], f32, tag="v")
                        for j in range(8):
                            nc.tensor.matmul(
                                ps[:], kvT[:, j, si * P:(si + 1) * P],
                                WvT[:, j, hl * HD:(hl + 1) * HD],
                                start=(j == 0), stop=(j == 7))
                        nc.vector.tensor_tensor(
                            v_aug[:, hl, si, 0:HD], ps[:], bv_rep[:, hl, :],
                            mybir.AluOpType.add)

            # ---------------- phase 2: scores + softmax + attn ----------------
            with tc.tile_pool(name="p2_bias", bufs=3) as bias_pool, \
                 tc.tile_pool(name="p2_pt", bufs=3) as pt_pool, \
                 tc.tile_pool(name="p2_sb", bufs=2) as p2_sb, \
                 tc.tile_pool(name="p2_sc", bufs=3, space="PSUM") as sc_pool, \
                 tc.tile_pool(name="p2_at", bufs=3, space="PSUM") as at_pool, \
                 tc.tile_pool(name="p2_tp", bufs=2, space="PSUM") as tp2_pool:

                def finish_block(hl, tb, at_ps):
                    # at_ps [65, 512]: rows 0..63 = attnT, row 64 = softmax denom
                    atT = p2_sb.tile([HD + 1, 512], f32, tag="atT")
                    nc.vector.tensor_copy(atT[:], at_ps[:])
                    for a in range(4):
                        tp = tp2_pool.tile([P, HD + 1], f32, tag="tp2")
                        nc.tensor.transpose(
                            tp[:], atT[:, a * P:(a + 1) * P],
                            id_f32[0:HD + 1, 0:HD + 1])
                        rec = p2_sb.tile([P, 1], f32, tag="rec")
                        nc.vector.reciprocal(rec[:], tp[:, HD:HD + 1])
                        an = p2_sb.tile([P, HD], bf16, tag="an")
                        nc.vector.tensor_scalar_mul(an[:], tp[:, 0:HD], rec[:])
                        nc.sync.dma_start(
                            scratch[hl, tb * 512 + a * P: tb * 512 + (a + 1) * P, :],
                            an[:])

                for hp in range(NH // 2):
                    for tb in range(TB):
                        tsl = slice(tb * 512, (tb + 1) * 512)
                        at_pss = [at_pool.tile([HD + 1, 512], f32, tag="at",
                                               name=f"at_{hp}_{tb}_{g2}")
                                  for g2 in range(2)]
                        for si in range(NSI):
                            for g in range(2):
                                hl = hp * 2 + g
                                gsl = halves[g]
                                bst = bias_pool.tile([P, 4, P], f32, tag="bst")
                                nc.sync.dma_start(
                                    bst[:],
                                    bias_p[hl, tsl, si * P:(si + 1) * P]
                                    .rearrange("(a p) s -> p a s", p=P))
                                sc_ps = sc_pool.tile([P, 512], f32, tag="sc")
                                for a in range(4):
                                    # one accumulation group for the whole bank:
                                    # start=True clears has_written bank-wide, so
                                    # only the first transpose may set it
                                    nc.tensor.matmul(
                                        sc_ps[:, a * P:(a + 1) * P], bst[:, a, :],
                                        id_f32[:], is_transpose=True,
                                        start=(a == 0), stop=False)
                                nc.tensor.matmul(
                                    sc_ps[:], kT[gsl, hp, si * P:(si + 1) * P],
                                    qT[gsl, hp, tsl], start=False, stop=True)
                                pt = pt_pool.tile([P, 512], bf16, tag="pt")
                                nc.scalar.activation(pt[:], sc_ps[:], Exp)
                                nc.tensor.matmul(
                                    at_pss[g], v_aug[:, hl, si, :], pt[:],
                                    start=(si == 0), stop=(si == NSI - 1))
                        for g in range(2):
                            finish_block(hp * 2 + g, tb, at_pss[g])

            # ---------------- phase 3: out projection ----------------
            with tc.tile_pool(name="p3_sb", bufs=2) as p3_sb, \
                 tc.tile_pool(name="p3_tp", bufs=2, space="PSUM") as tp3_pool, \
                 tc.tile_pool(name="p3_o", bufs=2, space="PSUM") as o_pool:
                for hl in range(NH):
                    A = p3_sb.tile([HD, E], bf16, tag="A")
                    nc.sync.dma_start(
                        A[:], scratch[hl].rearrange("(r x) d -> r (x d)", r=HD))
                    AT = p3_sb.tile([P, 8, HD], bf16, tag="AT")
                    for c in range(8):
                        tp = tp3_pool.tile([P, HD], bf16, tag="tp3")
                        nc.tensor.transpose(
                            tp[:], A[:, c * P:(c + 1) * P], id_bf[0:HD, 0:HD])
                        nc.vector.tensor_copy(AT[:, c, :], tp[:])
                    po = o_pool.tile([HD, E], f32, tag="po")
                    for n in range(2):
                        for c in range(8):
                            nc.tensor.matmul(
                                po[:, n * 512:(n + 1) * 512], AT[:, c, :],
                                WoT[:, c, n * 512:(n + 1) * 512],
                                start=(c == 0), stop=(c == 7))
                    ob = p3_sb.tile([HD, E], f32, tag="ob")
                    nc.vector.tensor_tensor(
                        ob[:], po[:], bo_rep[0:HD, :], mybir.AluOpType.add)
                    nc.sync.dma_start(out_p[hl * HD:(hl + 1) * HD, :], ob[:])

    nc.compile()
    return nc


def get_nc():
    if "nc" not in _cache:
        _cache["nc"] = _build_nc()
    return _cache["nc"]


def make_in_maps(inputs):
    f = lambda x: np.asarray(x, dtype=np.float32)
    hs = f(inputs["hidden_states"])
    kv = f(inputs["key_value_states"])
    bias = f(inputs["bias"])
    Wq, bq = f(inputs["Wq"]), f(inputs["bq"])
    Wk, bk = f(inputs["Wk"]), f(inputs["bk"])
    Wv, bv = f(inputs["Wv"]), f(inputs["bv"])
    Wo, bo = f(inputs["Wo"]), f(inputs["bo"])
    in_maps = []
    for c in range(NCORES):
        b, h0 = c // 2, (c % 2) * NH
        r = slice(h0 * HD, (h0 + NH) * HD)
        in_maps.append({
            "hs": hs[b], "kv": kv[b], "bias": bias[b, h0:h0 + NH],
            "Wq": Wq[r], "Wk": Wk[r], "Wv": Wv[r], "Wo": Wo,
            "bq": bq[r], "bk": bk[r], "bv": bv[r], "bo": bo,
        })
    return in_maps


def assemble(results):
    out = np.empty((B, T, E), dtype=np.float32)
    for c in range(NCORES):
        b, h0 = c // 2, (c % 2) * NH
        out[b, h0 * HD:(h0 + NH) * HD, :] = results[c]["out"]
    return out


def kernel(**inputs):
    from concourse.bass_utils import run_bass_kernel_spmd

    nc = get_nc()
    res = run_bass_kernel_spmd(nc, make_in_maps(inputs), core_ids=list(range(NCORES)))
    return assemble(res.results)


# revision 10
# speedup vs baseline: 61.3217x; 61.3217x over previous
"""Biased multi-head cross-attention on 8 TRN2 NeuronCores.

Math (per reference): q/k/v linear projections, scores = q@k^T/sqrt(hd) + bias,
softmax over source dim, attn = probs@v, then the "faithful" reshape
(B,H,T,hd)->(B,T,E) WITHOUT transposing heads back, followed by the out
projection. That reshape makes output rows [h*64,(h+1)*64) of each batch
depend only on head h, so the whole problem is embarrassingly parallel over
(batch, head): core c owns batch c//2 and heads (c%2)*8..(c%2)*8+8, computes
its 512 output rows, and the host concatenates. No collectives.

Per-core structure (pipelined over head PAIRS so the 64MB bias stream starts
early and overlaps the projection work of the next pair):
  prologue: transpose hs/kv/Wo once on PE (f32 tiles, batched PSUM->SBUF
    bf16 evacuations four 128x128 transposes per PSUM bank at a time).
  per pair hp (bufs=2 pipelining):
    - transpose this pair's Wq/Wk/Wv column slices
    - q^T/k^T projections packed two-heads-per-128-partitions; V projection
      packed [s,128] for the pair; V augmented with a ones column
    - scores: per (head, t-block 512, s-tile 128): bias tile DMA'd naturally
      [t,s], PE-transposed f32 straight into the PSUM bank (single
      accumulation group: only the first transpose uses start=True since
      start clears has_written bank-wide), then the K^T.Q matmul (K=64,
      even/odd heads on PE row-groups 0-63/64-127) accumulates on top.
      ACT exps PSUM -> SBUF bf16; attnT += v_aug.T @ pT accumulates in PSUM
      with row 64 = softmax denominator (ones trick).
    - attnT transposed back to [t,65], rows scaled by 1/denominator, bf16
      to DRAM scratch.
  P3: re-read scratch as [64,1024] rows (the faithful reshape is a pure
    memory view), PE-transpose into the stationary operand, matmul against
    WoT, add bo, store.
"""

import sys

for _p in ("/opt/trn_rl_repo", "/root/.axon_site/_ro/trn_rl_repo"):
    if _p not in sys.path:
        sys.path.insert(0, _p)

import numpy as np

B, T, S, E = 4, 1024, 2048, 1024
H, HD = 16, 64
NCORES = 8
NH = 8          # heads per core
P = 128
TB = T // 512   # t-blocks of 512
NSI = S // P    # s-tiles of 128
scaling = HD ** -0.5

_cache = {}


def _build_nc():
    import concourse.mybir as mybir
    import concourse.tile as tile
    from concourse import bacc
    from concourse.masks import make_identity

    f32 = mybir.dt.float32
    bf16 = mybir.dt.bfloat16
    Exp = mybir.ActivationFunctionType.Exp
    Ident = mybir.ActivationFunctionType.Identity

    nc = bacc.Bacc(None, target_bir_lowering=False)

    hs_p = nc.declare_dram_parameter("hs", [T, E], f32, isOutput=False)
    kv_p = nc.declare_dram_parameter("kv", [S, E], f32, isOutput=False)
    bias_p = nc.declare_dram_parameter("bias", [NH, T, S], f32, isOutput=False)
    Wq_p = nc.declare_dram_parameter("Wq", [NH * HD, E], f32, isOutput=False)
    Wk_p = nc.declare_dram_parameter("Wk", [NH * HD, E], f32, isOutput=False)
    Wv_p = nc.declare_dram_parameter("Wv", [NH * HD, E], f32, isOutput=False)
    Wo_p = nc.declare_dram_parameter("Wo", [E, E], f32, isOutput=False)
    bq_p = nc.declare_dram_parameter("bq", [NH * HD], f32, isOutput=False)
    bk_p = nc.declare_dram_parameter("bk", [NH * HD], f32, isOutput=False)
    bv_p = nc.declare_dram_parameter("bv", [NH * HD], f32, isOutput=False)
    bo_p = nc.declare_dram_parameter("bo", [E], f32, isOutput=False)
    out_p = nc.declare_dram_parameter("out", [NH * HD, E], f32, isOutput=True)

    halves = (slice(0, HD), slice(HD, P))

    with tile.TileContext(nc) as tc:
        with tc.tile_pool(name="statics", bufs=1) as statics, \
             tc.tile_pool(name="dram", bufs=1, space="DRAM") as dram:
            id_f32 = statics.tile([P, P], f32)
            make_identity(nc, id_f32[:])
            id_bf = statics.tile([P, P], bf16)
            make_identity(nc, id_bf[:])

            scratch = dram.tile([NH, T, HD], bf16)

            WoT = statics.tile([P, 8, E], bf16)
            bq_s = statics.tile([P, NH // 2], f32)   # bq_s[p,hp]=0.125*bq[hp*128+p]
            bk_s = statics.tile([P, NH // 2], f32)
            bv_rep = statics.tile([P, NH, HD], f32)
            bo_rep = statics.tile([P, E], f32)

            with tc.tile_pool(name="big", bufs=1) as big, \
                 tc.tile_pool(name="stage", bufs=2) as stage_pool, \
                 tc.tile_pool(name="pairw", bufs=2) as pairw, \
                 tc.tile_pool(name="pairqkv", bufs=2) as pairqkv, \
                 tc.tile_pool(name="p2sb", bufs=2) as p2sb, \
                 tc.tile_pool(name="bias", bufs=3) as bias_pool, \
                 tc.tile_pool(name="pt", bufs=6) as pt_pool, \
                 tc.tile_pool(name="tp", bufs=1, space="PSUM") as tp_pool, \
                 tc.tile_pool(name="proj", bufs=2, space="PSUM") as proj_pool, \
                 tc.tile_pool(name="sc", bufs=3, space="PSUM") as sc_pool, \
                 tc.tile_pool(name="at", bufs=2, space="PSUM") as at_pool:

                # ---- small vectors ----
                nc.sync.dma_start(bq_s[:], bq_p.rearrange("(hp p) -> p hp", p=P))
                nc.vector.tensor_scalar_mul(bq_s[:], bq_s[:], scaling)
                nc.sync.dma_start(bk_s[:], bk_p.rearrange("(hp p) -> p hp", p=P))
                bv_row = stage_pool.tile([1, NH * HD], f32, tag="vec")
                nc.sync.dma_start(bv_row[:], bv_p[None, :])
                for hl in range(NH):
                    nc.gpsimd.partition_broadcast(
                        bv_rep[:, hl, :], bv_row[0:1, hl * HD:(hl + 1) * HD])
                bo_row = stage_pool.tile([1, E], f32, tag="vec")
                nc.sync.dma_start(bo_row[:], bo_p[None, :])
                nc.gpsimd.partition_broadcast(bo_rep[:], bo_row[0:1, :])

                def transpose_in(dst, src_ap, nrow_tiles, row0=0):
                    # src [nrow_tiles*128, E] f32 -> dst[:, j, i*128...] bf16
                    # (E on partitions), batched evacuation 4 chunks per bank
                    for i in range(nrow_tiles):
                        st = stage_pool.tile([P, E], f32, tag="st")
                        r = row0 + i * P
                        nc.sync.dma_start(st[:], src_ap[r:r + P, :])
                        for jb in range(2):
                            tp = tp_pool.tile([P, 4, P], f32, tag="tp")
                            for a in range(4):
                                nc.tensor.matmul(
                                    tp[:, a, :], st[:, (jb * 4 + a) * P:(jb * 4 + a + 1) * P],
                                    id_f32[:], is_transpose=True,
                                    start=(a == 0), stop=(a == 3))
                            nc.vector.tensor_copy(
                                dst[:, jb * 4:(jb + 1) * 4, i * P:(i + 1) * P], tp[:])

                hsT = big.tile([P, 8, T], bf16)
                kvT = big.tile([P, 8, S], bf16)
                transpose_in(hsT, hs_p, T // P)
                transpose_in(kvT, kv_p, S // P)
                transpose_in(WoT, Wo_p, E // P)

                for hp in range(NH // 2):
                    # ---- this pair's weight slices, transposed ----
                    WqTp = pairw.tile([P, 8, P], bf16, tag="wq")
                    WkTp = pairw.tile([P, 8, P], bf16, tag="wk")
                    WvTp = pairw.tile([P, 8, P], bf16, tag="wv")
                    for dst, src in ((WqTp, Wq_p), (WkTp, Wk_p), (WvTp, Wv_p)):
                        st = stage_pool.tile([P, E], f32, tag="st")
                        nc.sync.dma_start(st[:], src[hp * P:(hp + 1) * P, :])
                        for jb in range(2):
                            tp = tp_pool.tile([P, 4, P], f32, tag="tp")
                            for a in range(4):
                                nc.tensor.matmul(
                                    tp[:, a, :], st[:, (jb * 4 + a) * P:(jb * 4 + a + 1) * P],
                                    id_f32[:], is_transpose=True,
                                    start=(a == 0), stop=(a == 3))
                            nc.vector.tensor_copy(
                                dst[:, jb * 4:(jb + 1) * 4, :], tp[:])

                    # ---- projections for the pair ----
                    qTp = pairqkv.tile([P, T], bf16, tag="qTp")
                    kTp = pairqkv.tile([P, S], bf16, tag="kTp")
                    v_aug = pairqkv.tile([P, 2, NSI, HD + 1], bf16, tag="vaug")
                    nc.any.memset(v_aug[:, :, :, HD:HD + 1], 1.0)
                    for tb in range(TB):
                        ps = proj_pool.tile([P, 512], f32, tag="proj")
                        for g in range(2):
                            for j in range(8):
                                nc.tensor.matmul(
                                    ps[halves[g], :], WqTp[:, j, g * HD:(g + 1) * HD],
                                    hsT[:, j, tb * 512:(tb + 1) * 512],
                                    start=(j == 0), stop=(j == 7))
                            nc.scalar.activation(
                                qTp[halves[g], tb * 512:(tb + 1) * 512],
                                ps[halves[g], :], Ident,
                                bias=bq_s[halves[g], hp:hp + 1], scale=scaling)
                    for sb in range(S // 512):
                        ps = proj_pool.tile([P, 512], f32, tag="proj")
                        for g in range(2):
                            for j in range(8):
                                nc.tensor.matmul(
                                    ps[halves[g], :], WkTp[:, j, g * HD:(g + 1) * HD],
                                    kvT[:, j, sb * 512:(sb + 1) * 512],
                                    start=(j == 0), stop=(j == 7))
                            nc.scalar.activation(
                                kTp[halves[g], sb * 512:(sb + 1) * 512],
                                ps[halves[g], :], Ident,
                                bias=bk_s[halves[g], hp:hp + 1])
                    for si in range(NSI):
                        ps = proj_pool.tile([P, P], f32, tag="proj")
                        for j in range(8):
                            nc.tensor.matmul(
                                ps[:], kvT[:, j, si * P:(si + 1) * P], WvTp[:, j, :],
                                start=(j == 0), stop=(j == 7))
                        for g in range(2):
                            nc.vector.tensor_tensor(
                                v_aug[:, g, si, 0:HD], ps[:, g * HD:(g + 1) * HD],
                                bv_rep[:, hp * 2 + g, :], mybir.AluOpType.add)

                    # ---- scores + softmax + attn for the pair ----
                    for tb in range(TB):
                        tsl = slice(tb * 512, (tb + 1) * 512)
                        at_pss = [at_pool.tile([HD + 1, 512], f32, tag="at",
                                               name=f"at_{hp}_{tb}_{g2}")
                                  for g2 in range(2)]
                        pend = []
                        for sq in range(NSI // 4):
                            bsts = []
                            for g in range(2):
                                hl = hp * 2 + g
                                bst = bias_pool.tile([P, 4, 4 * P], f32, tag="bst",
                                                     name=f"bst_{hp}_{tb}_{sq}_{g}")
                                nc.sync.dma_start(
                                    bst[:],
                                    bias_p[hl, tsl, sq * 4 * P:(sq + 1) * 4 * P]
                                    .rearrange("(a p) s -> p a s", p=P))
                                bsts.append(bst)
                            for sisub in range(4):
                                si = sq * 4 + sisub
                                for g in range(2):
                                    gsl = halves[g]
                                    sc_ps = sc_pool.tile([P, 512], f32, tag="sc")
                                    for a in range(4):
                                        nc.tensor.matmul(
                                            sc_ps[:, a * P:(a + 1) * P],
                                            bsts[g][:, a, sisub * P:(sisub + 1) * P],
                                            id_f32[:], is_transpose=True,
                                            start=(a == 0), stop=False)
                                    nc.tensor.matmul(
                                        sc_ps[:], kTp[gsl, si * P:(si + 1) * P],
                                        qTp[gsl, tsl], start=False, stop=True)
                                    pt = pt_pool.tile([P, 512], bf16, tag="pt")
                                    nc.scalar.activation(pt[:], sc_ps[:], Exp)
                                    pend.append((g, si, pt))
                                # drain attn matmuls one si behind so PE's
                                # strict FIFO never waits on ACT's exp
                                while len(pend) > 2:
                                    g2, si2, pt2 = pend.pop(0)
                                    nc.tensor.matmul(
                                        at_pss[g2], v_aug[:, g2, si2, :], pt2[:],
                                        start=(si2 == 0), stop=(si2 == NSI - 1))
                        for g2, si2, pt2 in pend:
                            nc.tensor.matmul(
                                at_pss[g2], v_aug[:, g2, si2, :], pt2[:],
                                start=(si2 == 0), stop=(si2 == NSI - 1))
                        pend = []
                        for g in range(2):
                            hl = hp * 2 + g
                            atT = p2sb.tile([HD + 1, 512], f32, tag="atT")
                            nc.vector.tensor_copy(atT[:], at_pss[g])
                            for a in range(4):
                                tp = tp_pool.tile([P, HD + 1], f32, tag="tp")
                                nc.tensor.transpose(
                                    tp[:], atT[:, a * P:(a + 1) * P],
                                    id_f32[0:HD + 1, 0:HD + 1])
                                rec = p2sb.tile([P, 1], f32, tag="rec")
                                nc.vector.reciprocal(rec[:], tp[:, HD:HD + 1])
                                an = p2sb.tile([P, HD], bf16, tag="an")
                                nc.vector.tensor_scalar_mul(an[:], tp[:, 0:HD], rec[:])
                                nc.sync.dma_start(
                                    scratch[hl, tb * 512 + a * P: tb * 512 + (a + 1) * P, :],
                                    an[:])

            # ---------------- phase 3: out projection ----------------
            with tc.tile_pool(name="p3_sb", bufs=2) as p3_sb, \
                 tc.tile_pool(name="p3_tp", bufs=2, space="PSUM") as tp3_pool, \
                 tc.tile_pool(name="p3_o", bufs=2, space="PSUM") as o_pool:
                for hl in range(NH):
                    A = p3_sb.tile([HD, E], bf16, tag="A")
                    nc.sync.dma_start(
                        A[:], scratch[hl].rearrange("(r x) d -> r (x d)", r=HD))
                    AT = p3_sb.tile([P, 8, HD], bf16, tag="AT")
                    for c in range(8):
                        tp = tp3_pool.tile([P, HD], bf16, tag="tp3")
                        nc.tensor.transpose(
                            tp[:], A[:, c * P:(c + 1) * P], id_bf[0:HD, 0:HD])
                        nc.vector.tensor_copy(AT[:, c, :], tp[:])
                    po = o_pool.tile([HD, E], f32, tag="po")
                    for n in range(2):
                        for c in range(8):
                            nc.tensor.matmul(
                                po[:, n * 512:(n + 1) * 512], AT[:, c, :],
                                WoT[:, c, n * 512:(n + 1) * 512],
                                start=(c == 0), stop=(c == 7))
                    ob = p3_sb.tile([HD, E], f32, tag="ob")
                    nc.vector.tensor_tensor(
                        ob[:], po[:], bo_rep[0:HD, :], mybir.AluOpType.add)
                    nc.sync.dma_start(out_p[hl * HD:(hl + 1) * HD, :], ob[:])

    nc.compile()
    return nc


def get_nc():
    if "nc" not in _cache:
        _cache["nc"] = _build_nc()
    return _cache["nc"]


def make_in_maps(inputs):
    f = lambda x: np.asarray(x, dtype=np.float32)
    hs = f(inputs["hidden_states"])
    kv = f(inputs["key_value_states"])
    bias = f(inputs["bias"])
    Wq, bq = f(inputs["Wq"]), f(inputs["bq"])
    Wk, bk = f(inputs["Wk"]), f(inputs["bk"])
    Wv, bv = f(inputs["Wv"]), f(inputs["bv"])
    Wo, bo = f(inputs["Wo"]), f(inputs["bo"])
    in_maps = []
    for c in range(NCORES):
        b, h0 = c // 2, (c % 2) * NH
        r = slice(h0 * HD, (h0 + NH) * HD)
        in_maps.append({
            "hs": hs[b], "kv": kv[b], "bias": bias[b, h0:h0 + NH],
            "Wq": Wq[r], "Wk": Wk[r], "Wv": Wv[r], "Wo": Wo,
            "bq": bq[r], "bk": bk[r], "bv": bv[r], "bo": bo,
        })
    return in_maps


def assemble(results):
    out = np.empty((B, T, E), dtype=np.float32)
    for c in range(NCORES):
        b, h0 = c // 2, (c % 2) * NH
        out[b, h0 * HD:(h0 + NH) * HD, :] = results[c]["out"]
    return out


def kernel(**inputs):
    from concourse.bass_utils import run_bass_kernel_spmd

    nc = get_nc()
    res = run_bass_kernel_spmd(nc, make_in_maps(inputs), core_ids=list(range(NCORES)))
    return assemble(res.results)


# revision 14
# speedup vs baseline: 123.7575x; 2.0182x over previous
"""Biased multi-head cross-attention on 8 TRN2 NeuronCores.

Math (per reference): q/k/v linear projections, scores = q@k^T/sqrt(hd) + bias,
softmax over source dim, attn = probs@v, then the "faithful" reshape
(B,H,T,hd)->(B,T,E) WITHOUT transposing heads back, followed by the out
projection. That reshape makes output rows [h*64,(h+1)*64) of each batch
depend only on head h, so the whole problem is embarrassingly parallel over
(batch, head): core c owns batch c//2 and heads (c%2)*8..(c%2)*8+8, computes
its 512 output rows, and the host concatenates. No collectives.

Per-core structure (pipelined over head PAIRS so the 64MB bias stream starts
early and overlaps the projection work of the next pair):
  prologue: transpose hs/kv/Wo once on PE (f32 tiles, batched PSUM->SBUF
    bf16 evacuations four 128x128 transposes per PSUM bank at a time).
  per pair hp (bufs=2 pipelining):
    - transpose this pair's Wq/Wk/Wv column slices
    - q^T/k^T projections packed two-heads-per-128-partitions; V projection
      packed [s,128] for the pair; V augmented with a ones column
    - scores: per (head, t-block 512, s-tile 128): bias tile DMA'd naturally
      [t,s], PE-transposed f32 straight into the PSUM bank (single
      accumulation group: only the first transpose uses start=True since
      start clears has_written bank-wide), then the K^T.Q matmul (K=64,
      even/odd heads on PE row-groups 0-63/64-127) accumulates on top.
      ACT exps PSUM -> SBUF bf16; attnT += v_aug.T @ pT accumulates in PSUM
      with row 64 = softmax denominator (ones trick).
    - attnT transposed back to [t,65], rows scaled by 1/denominator, bf16
      to DRAM scratch.
  P3: re-read scratch as [64,1024] rows (the faithful reshape is a pure
    memory view), PE-transpose into the stationary operand, matmul against
    WoT, add bo, store.
"""

import sys

for _p in ("/opt/trn_rl_repo", "/root/.axon_site/_ro/trn_rl_repo"):
    if _p not in sys.path:
        sys.path.insert(0, _p)

import numpy as np

B, T, S, E = 4, 1024, 2048, 1024
H, HD = 16, 64
NCORES = 8
NH = 8          # heads per core
P = 128
TB = T // 512   # t-blocks of 512
NSI = S // P    # s-tiles of 128
scaling = HD ** -0.5

_cache = {}


def _build_nc():
    import concourse.mybir as mybir
    import concourse.tile as tile
    from concourse import bacc
    from concourse.masks import make_identity

    f32 = mybir.dt.float32
    bf16 = mybir.dt.bfloat16
    Exp = mybir.ActivationFunctionType.Exp
    Ident = mybir.ActivationFunctionType.Identity

    nc = bacc.Bacc(None, target_bir_lowering=False)

    hs_p = nc.declare_dram_parameter("hs", [T, E], f32, isOutput=False)
    kv_p = nc.declare_dram_parameter("kv", [S, E], f32, isOutput=False)
    bias_p = nc.declare_dram_parameter("bias", [NH, T, S], f32, isOutput=False)
    Wq_p = nc.declare_dram_parameter("Wq", [NH * HD, E], f32, isOutput=False)
    Wk_p = nc.declare_dram_parameter("Wk", [NH * HD, E], f32, isOutput=False)
    Wv_p = nc.declare_dram_parameter("Wv", [NH * HD, E], f32, isOutput=False)
    Wo_p = nc.declare_dram_parameter("Wo", [E, E], f32, isOutput=False)
    bq_p = nc.declare_dram_parameter("bq", [NH * HD], f32, isOutput=False)
    bk_p = nc.declare_dram_parameter("bk", [NH * HD], f32, isOutput=False)
    bv_p = nc.declare_dram_parameter("bv", [NH * HD], f32, isOutput=False)
    bo_p = nc.declare_dram_parameter("bo", [E], f32, isOutput=False)
    out_p = nc.declare_dram_parameter("out", [NH * HD, E], f32, isOutput=True)

    halves = (slice(0, HD), slice(HD, P))

    with tile.TileContext(nc) as tc:
        with tc.tile_pool(name="statics", bufs=1) as statics, \
             tc.tile_pool(name="dram", bufs=1, space="DRAM") as dram:
            id_f32 = statics.tile([P, P], f32)
            make_identity(nc, id_f32[:])
            id_bf = statics.tile([P, P], bf16)
            make_identity(nc, id_bf[:])

            scratch = dram.tile([NH, T, HD], bf16)

            WoT = statics.tile([P, 8, E], bf16)
            bq_s = statics.tile([P, NH // 2], f32)   # bq_s[p,hp]=0.125*bq[hp*128+p]
            bk_s = statics.tile([P, NH // 2], f32)
            bv_rep = statics.tile([P, NH, HD], f32)
            bo_rep = statics.tile([P, E], f32)

            with tc.tile_pool(name="big", bufs=1) as big, \
                 tc.tile_pool(name="stage", bufs=2) as stage_pool, \
                 tc.tile_pool(name="pairw", bufs=2) as pairw, \
                 tc.tile_pool(name="pairqkv", bufs=2) as pairqkv, \
                 tc.tile_pool(name="p2sb", bufs=2) as p2sb, \
                 tc.tile_pool(name="bias", bufs=5) as bias_pool, \
                 tc.tile_pool(name="pt", bufs=8) as pt_pool, \
                 tc.tile_pool(name="tp", bufs=1, space="PSUM") as tp_pool, \
                 tc.tile_pool(name="proj", bufs=2, space="PSUM") as proj_pool, \
                 tc.tile_pool(name="sc", bufs=3, space="PSUM") as sc_pool, \
                 tc.tile_pool(name="at", bufs=2, space="PSUM") as at_pool:

                # ---- small vectors ----
                nc.sync.dma_start(bq_s[:], bq_p.rearrange("(hp p) -> p hp", p=P))
                nc.vector.tensor_scalar_mul(bq_s[:], bq_s[:], scaling)
                nc.sync.dma_start(bk_s[:], bk_p.rearrange("(hp p) -> p hp", p=P))
                bv_row = stage_pool.tile([1, NH * HD], f32, tag="vec")
                nc.sync.dma_start(bv_row[:], bv_p[None, :])
                for hl in range(NH):
                    nc.gpsimd.partition_broadcast(
                        bv_rep[:, hl, :], bv_row[0:1, hl * HD:(hl + 1) * HD])
                bo_row = stage_pool.tile([1, E], f32, tag="vec")
                nc.sync.dma_start(bo_row[:], bo_p[None, :])
                nc.gpsimd.partition_broadcast(bo_rep[:], bo_row[0:1, :])

                def transpose_in(dst, src_ap, nrow_tiles, row0=0):
                    # src [nrow_tiles*128, E] f32 -> dst[:, j, i*128...] bf16
                    # (E on partitions), batched evacuation 4 chunks per bank
                    for i in range(nrow_tiles):
                        st = stage_pool.tile([P, E], f32, tag="st")
                        r = row0 + i * P
                        nc.sync.dma_start(st[:], src_ap[r:r + P, :])
                        for jb in range(2):
                            tp = tp_pool.tile([P, 4, P], f32, tag="tp")
                            for a in range(4):
                                nc.tensor.matmul(
                                    tp[:, a, :], st[:, (jb * 4 + a) * P:(jb * 4 + a + 1) * P],
                                    id_f32[:], is_transpose=True,
                                    start=(a == 0), stop=(a == 3))
                            nc.vector.tensor_copy(
                                dst[:, jb * 4:(jb + 1) * 4, i * P:(i + 1) * P], tp[:])

                hsT = big.tile([P, 8, T], bf16)
                kvT = big.tile([P, 8, S], bf16)
                transpose_in(hsT, hs_p, T // P)
                transpose_in(kvT, kv_p, S // P)
                transpose_in(WoT, Wo_p, E // P)

                for hp in range(NH // 2):
                    # ---- this pair's weight slices, transposed ----
                    WqTp = pairw.tile([P, 8, P], bf16, tag="wq")
                    WkTp = pairw.tile([P, 8, P], bf16, tag="wk")
                    WvTp = pairw.tile([P, 8, P], bf16, tag="wv")
                    for dst, src in ((WqTp, Wq_p), (WkTp, Wk_p), (WvTp, Wv_p)):
                        st = stage_pool.tile([P, E], f32, tag="st")
                        nc.sync.dma_start(st[:], src[hp * P:(hp + 1) * P, :])
                        for jb in range(2):
                            tp = tp_pool.tile([P, 4, P], f32, tag="tp")
                            for a in range(4):
                                nc.tensor.matmul(
                                    tp[:, a, :], st[:, (jb * 4 + a) * P:(jb * 4 + a + 1) * P],
                                    id_f32[:], is_transpose=True,
                                    start=(a == 0), stop=(a == 3))
                            nc.vector.tensor_copy(
                                dst[:, jb * 4:(jb + 1) * 4, :], tp[:])

                    # ---- projections for the pair ----
                    qTp = pairqkv.tile([P, T], bf16, tag="qTp")
                    kTp = pairqkv.tile([P, S], bf16, tag="kTp")
                    v_aug = pairqkv.tile([P, 2, NSI, HD + 1], bf16, tag="vaug")
                    nc.any.memset(v_aug[:, :, :, HD:HD + 1], 1.0)
                    for tb in range(TB):
                        ps = proj_pool.tile([P, 512], f32, tag="proj")
                        for g in range(2):
                            for j in range(8):
                                nc.tensor.matmul(
                                    ps[halves[g], :], WqTp[:, j, g * HD:(g + 1) * HD],
                                    hsT[:, j, tb * 512:(tb + 1) * 512],
                                    start=(j == 0), stop=(j == 7))
                            nc.scalar.activation(
                                qTp[halves[g], tb * 512:(tb + 1) * 512],
                                ps[halves[g], :], Ident,
                                bias=bq_s[halves[g], hp:hp + 1], scale=scaling)
                    for sb in range(S // 512):
                        ps = proj_pool.tile([P, 512], f32, tag="proj")
                        for g in range(2):
                            for j in range(8):
                                nc.tensor.matmul(
                                    ps[halves[g], :], WkTp[:, j, g * HD:(g + 1) * HD],
                                    kvT[:, j, sb * 512:(sb + 1) * 512],
                                    start=(j == 0), stop=(j == 7))
                            nc.scalar.activation(
                                kTp[halves[g], sb * 512:(sb + 1) * 512],
                                ps[halves[g], :], Ident,
                                bias=bk_s[halves[g], hp:hp + 1])
                    for si in range(NSI):
                        ps = proj_pool.tile([P, P], f32, tag="proj")
                        for j in range(8):
                            nc.tensor.matmul(
                                ps[:], kvT[:, j, si * P:(si + 1) * P], WvTp[:, j, :],
                                start=(j == 0), stop=(j == 7))
                        for g in range(2):
                            nc.vector.tensor_tensor(
                                v_aug[:, g, si, 0:HD], ps[:, g * HD:(g + 1) * HD],
                                bv_rep[:, hp * 2 + g, :], mybir.AluOpType.add)

                    # ---- scores + softmax + attn for the pair ----
                    for tb in range(TB):
                        tsl = slice(tb * 512, (tb + 1) * 512)
                        at_pss = [at_pool.tile([HD + 1, 512], f32, tag="at",
                                               name=f"at_{hp}_{tb}_{g2}")
                                  for g2 in range(2)]
                        pend = []
                        for sq in range(NSI // 4):
                            bsts = []
                            for g in range(2):
                                hl = hp * 2 + g
                                bst = bias_pool.tile([P, 4, 4 * P], f32, tag="bst",
                                                     name=f"bst_{hp}_{tb}_{sq}_{g}")
                                nc.sync.dma_start(
                                    bst[:],
                                    bias_p[hl, tsl, sq * 4 * P:(sq + 1) * 4 * P]
                                    .rearrange("(a p) s -> p a s", p=P))
                                bsts.append(bst)
                            for sisub in range(4):
                                si = sq * 4 + sisub
                                for g in range(2):
                                    gsl = halves[g]
                                    sc_ps = sc_pool.tile([P, 512], f32, tag="sc")
                                    for a in range(4):
                                        nc.tensor.matmul(
                                            sc_ps[:, a * P:(a + 1) * P],
                                            bsts[g][:, a, sisub * P:(sisub + 1) * P],
                                            id_f32[:], is_transpose=True,
                                            start=(a == 0), stop=False)
                                    nc.tensor.matmul(
                                        sc_ps[:], kTp[gsl, si * P:(si + 1) * P],
                                        qTp[gsl, tsl], start=False, stop=True)
                                    pt = pt_pool.tile([P, 512], bf16, tag="pt")
                                    nc.scalar.activation(pt[:], sc_ps[:], Exp)
                                    pend.append((g, si, pt))
                                # drain attn matmuls one si behind so PE's
                                # strict FIFO never waits on ACT's exp
                                while len(pend) > 2:
                                    g2, si2, pt2 = pend.pop(0)
                                    nc.tensor.matmul(
                                        at_pss[g2], v_aug[:, g2, si2, :], pt2[:],
                                        start=(si2 == 0), stop=(si2 == NSI - 1))
                        for g2, si2, pt2 in pend:
                            nc.tensor.matmul(
                                at_pss[g2], v_aug[:, g2, si2, :], pt2[:],
                                start=(si2 == 0), stop=(si2 == NSI - 1))
                        pend = []
                        for g in range(2):
                            hl = hp * 2 + g
                            atT = p2sb.tile([HD + 1, 512], f32, tag="atT")
                            nc.vector.tensor_copy(atT[:], at_pss[g])
                            for a in range(4):
                                tp = tp_pool.tile([P, HD + 1], f32, tag="tp")
                                nc.tensor.transpose(
                                    tp[:], atT[:, a * P:(a + 1) * P],
                                    id_f32[0:HD + 1, 0:HD + 1])
                                rec = p2sb.tile([P, 1], f32, tag="rec")
                                nc.vector.reciprocal(rec[:], tp[:, HD:HD + 1])
                                an = p2sb.tile([P, HD], bf16, tag="an")
                                nc.vector.tensor_scalar_mul(an[:], tp[:, 0:HD], rec[:])
                                nc.sync.dma_start(
                                    scratch[hl, tb * 512 + a * P: tb * 512 + (a + 1) * P, :],
                                    an[:])

            # ---------------- phase 3: out projection ----------------
            with tc.tile_pool(name="p3_sb", bufs=2) as p3_sb, \
                 tc.tile_pool(name="p3_tp", bufs=2, space="PSUM") as tp3_pool, \
                 tc.tile_pool(name="p3_o", bufs=2, space="PSUM") as o_pool:
                for hl in range(NH):
                    A = p3_sb.tile([HD, E], bf16, tag="A")
                    nc.sync.dma_start(
                        A[:], scratch[hl].rearrange("(r x) d -> r (x d)", r=HD))
                    AT = p3_sb.tile([P, 8, HD], bf16, tag="AT")
                    for c in range(8):
                        tp = tp3_pool.tile([P, HD], bf16, tag="tp3")
                        nc.tensor.transpose(
                            tp[:], A[:, c * P:(c + 1) * P], id_bf[0:HD, 0:HD])
                        nc.vector.tensor_copy(AT[:, c, :], tp[:])
                    po = o_pool.tile([HD, E], f32, tag="po")
                    for n in range(2):
                        for c in range(8):
                            nc.tensor.matmul(
                                po[:, n * 512:(n + 1) * 512], AT[:, c, :],
                                WoT[:, c, n * 512:(n + 1) * 512],
                                start=(c == 0), stop=(c == 7))
                    ob = p3_sb.tile([HD, E], f32, tag="ob")
                    nc.vector.tensor_tensor(
                        ob[:], po[:], bo_rep[0:HD, :], mybir.AluOpType.add)
                    nc.sync.dma_start(out_p[hl * HD:(hl + 1) * HD, :], ob[:])

    nc.compile()
    return nc


def get_nc():
    if "nc" not in _cache:
        _cache["nc"] = _build_nc()
    return _cache["nc"]


def make_in_maps(inputs):
    f = lambda x: np.asarray(x, dtype=np.float32)
    hs = f(inputs["hidden_states"])
    kv = f(inputs["key_value_states"])
    bias = f(inputs["bias"])
    Wq, bq = f(inputs["Wq"]), f(inputs["bq"])
    Wk, bk = f(inputs["Wk"]), f(inputs["bk"])
    Wv, bv = f(inputs["Wv"]), f(inputs["bv"])
    Wo, bo = f(inputs["Wo"]), f(inputs["bo"])
    in_maps = []
    for c in range(NCORES):
        b, h0 = c // 2, (c % 2) * NH
        r = slice(h0 * HD, (h0 + NH) * HD)
        in_maps.append({
            "hs": hs[b], "kv": kv[b], "bias": bias[b, h0:h0 + NH],
            "Wq": Wq[r], "Wk": Wk[r], "Wv": Wv[r], "Wo": Wo,
            "bq": bq[r], "bk": bk[r], "bv": bv[r], "bo": bo,
        })
    return in_maps


def assemble(results):
    out = np.empty((B, T, E), dtype=np.float32)
    for c in range(NCORES):
        b, h0 = c // 2, (c % 2) * NH
        out[b, h0 * HD:(h0 + NH) * HD, :] = results[c]["out"]
    return out


def kernel(**inputs):
    from concourse.bass_utils import run_bass_kernel_spmd

    nc = get_nc()
    res = run_bass_kernel_spmd(nc, make_in_maps(inputs), core_ids=list(range(NCORES)))
    return assemble(res.results)
